# revision 37
# baseline (speedup 1.0000x reference)
"""GQA attention block (qk-rmsnorm + RoPE + causal GQA attention + out-proj),
tensor-parallel over 8 NeuronCores: 2-way data parallel (batch) x 4-way head
parallel (8 q heads / 2 kv heads per core). All-reduce of out-proj partials is
done on host (sum of 4 partials per batch).

Per-core layouts (device):
  phase 1: q/k/v projections with x^T chunks stationary on PE -> [T,d] rows;
           qk-rmsnorm + RoPE in row layout (f16); PE-transpose q,k to [d,T].
           Tiles 0-7 run up front; tiles 8-15 are sprinkled into the phase-2
           job stream (block i only needs tiles 0..4i+3) to fill PE stalls.
  phase 2: per (head-chunk c, key chunk j): S^T for BOTH kv heads back-to-back
           (K=64 matmuls on disjoint PE row halves, hardware-concurrent),
           one exp covering both (scale=1/8 + ln(1/64) folded) -> P~ f16;
           causal triangle applied as a 0/1 f16 mask multiply on DVE; PV via
           [V|1] stationary -> out^T + rowsum in one psum; DVE fast-reciprocal
           + f16 PE row-broadcast.
  phase 3: out-proj from packed head pairs, f16 weights, accumulate f32 psum,
           f16 partial outputs summed on host.
"""
import sys
import numpy as np

sys.path.insert(0, "/opt/trn_rl_repo")

import concourse.bass as bass  # noqa: E402
import concourse.bacc as bacc  # noqa: E402
import concourse.mybir as mybir  # noqa: E402
import concourse.tile as tile  # noqa: E402
from concourse import masks  # noqa: E402
from concourse.bass_utils import run_bass_kernel_spmd  # noqa: E402

f32 = mybir.dt.float32
f32r = mybir.dt.float32r
f16 = mybir.dt.float16
FT = mybir.ActivationFunctionType
AX = mybir.AxisListType

P = 128
T = 2048
H = 2048
D = 64
NQ = 8          # q heads per core
DQ = NQ * D     # 512
NTT = T // P    # 16 T tiles
NHC = H // P    # 16 hidden chunks
NBLK = 4        # T_q blocks of 512
BLK = 512
EPS = 1e-5
LN64 = -4.1588830833596715  # ln(1/64): scales exp to keep 1/rowsum in f16 normal range

_CACHE = {}


def _build_program():
    nc = bacc.Bacc("TRN2", target_bir_lowering=False, debug=False, num_devices=8)

    xT_d = nc.dram_tensor("xT", [H, T], f16, kind="ExternalInput")
    wqkv_d = nc.dram_tensor("wqkv", [H, 768], f16, kind="ExternalInput")
    wo_d = nc.dram_tensor("wo", [DQ, H], f16, kind="ExternalInput")
    ropeq_d = nc.dram_tensor("ropeq", [T, 1024], f16, kind="ExternalInput")
    ropek_d = nc.dram_tensor("ropek", [T, 128], f16, kind="ExternalInput")
    mtab_d = nc.dram_tensor("mtab", [P, 128], f16, kind="ExternalInput")
    out_d = nc.dram_tensor("out", [T, H], f16, kind="ExternalOutput")

    with tile.TileContext(nc) as tc:
        with (
            tc.tile_pool(name="persist", bufs=1) as pp,
            tc.tile_pool(name="work2", bufs=2) as wp,
            tc.tile_pool(name="ptp", bufs=3) as ptp,
            tc.tile_pool(name="obp", bufs=8) as obp,
            tc.tile_pool(name="psum", bufs=3, space="PSUM") as ps,
            tc.tile_pool(name="psum_o", bufs=2, space="PSUM") as pso,
            tc.tile_pool(name="ph1", bufs=1) as p1,
            tc.tile_pool(name="work1", bufs=2) as wp1,
        ):
            # ---------- long-lived tiles ----------
            wo_sb = []
            for c in range(4):
                t3 = pp.tile([P, H], f16, tag=f"wo{c}")
                nc.sync.dma_start(t3[:], wo_d[c * P:(c + 1) * P, :])
                wo_sb.append(t3)
            tri = pp.tile([P, P], f16, tag="tri")   # 1 where key<=query (p<=c)
            nc.sync.dma_start(tri[:], mtab_d[:])
            ones = pp.tile([P, 65], f16, tag="ones")
            nc.gpsimd.memset(ones[:], 1.0)
            lnb = pp.tile([P, 1], f32, tag="lnb")
            nc.gpsimd.memset(lnb[:], LN64)
            epsb = pp.tile([P, 1], f32, tag="epsb")
            nc.gpsimd.memset(epsb[:], EPS)

            # qT stored tile-major: [d-pair(128), tt(16) x c(4) x t(128)]
            qT = pp.tile([P, NTT * 512], f16, tag="qT")
            kT = pp.tile([P, T], f16, tag="kT")        # kv0 rows 0:64, kv1 rows 64:128
            vsb = []
            for tt in range(NTT):
                vt = pp.tile([P, 130], f16, tag=f"v{tt}")
                nc.gpsimd.memset(vt[:, 64:65], 1.0)     # ones col for kv0
                nc.gpsimd.memset(vt[:, 129:130], 1.0)   # ones col for kv1
                vsb.append(vt)

            qTv = qT[:].rearrange("p (tt c t) -> p tt c t", c=4, t=P)

            # ---------- phase-1 inputs ----------
            wqkv_sb = []
            xt_sb = []
            for hc in range(NHC):
                t1 = p1.tile([P, 768], f16, tag=f"wqkv{hc}")
                nc.sync.dma_start(t1[:], wqkv_d[hc * P:(hc + 1) * P, :])
                wqkv_sb.append(t1)
                tx = p1.tile([P, T], f16, tag=f"xt{hc}")
                nc.sync.dma_start(tx[:], xT_d[hc * P:(hc + 1) * P, :])
                xt_sb.append(tx)
            ropeq_sb = p1.tile([P, 16 * 1024], f16, tag="ropeq")
            ropek_sb = p1.tile([P, 16 * 128], f16, tag="ropek")
            for tt in range(NTT):
                nc.sync.dma_start(ropeq_sb[:, tt * 1024:(tt + 1) * 1024],
                                  ropeq_d[tt * P:(tt + 1) * P, :])
                nc.sync.dma_start(ropek_sb[:, tt * 128:(tt + 1) * 128],
                                  ropek_d[tt * P:(tt + 1) * P, :])
            ident = p1.tile([P, P], f16, tag="ident")
            masks.make_identity(nc, ident[:])

            ropes = {}   # tt -> (qrope, krope) awaiting transpose

            def emit_tile(tt):
                pa = ps.tile([P, 1024], f32, tag="a")
                for hc in range(NHC):
                    lhs = xt_sb[hc][:, tt * P:(tt + 1) * P]
                    nc.tensor.matmul(pa[:, 0:512], lhs, wqkv_sb[hc][:, 0:512],
                                     start=(hc == 0), stop=(hc == NHC - 1))
                    nc.tensor.matmul(pa[:, 512:768], lhs, wqkv_sb[hc][:, 512:768],
                                     start=(hc == 0), stop=(hc == NHC - 1))
                # v eviction (no norm): two flat copies into the kv slots
                vt = vsb[tt]
                nc.vector.tensor_copy(vt[:, 0:64], pa[:, 640:704])
                nc.vector.tensor_copy(vt[:, 65:129], pa[:, 704:768])
                # q+k rmsnorm (one fused square over q|k columns)
                sq = wp1.tile([P, 640], f32, tag="sq")
                nc.scalar.activation(sq[:], pa[:, 0:640], FT.Square)
                red = wp1.tile([P, NQ], f32, tag="red")
                nc.vector.reduce_sum(red[:].unsqueeze(-1),
                                     sq[:, 0:512].rearrange("p (h d) -> p h d", d=D),
                                     axis=AX.X)
                srt = wp1.tile([P, NQ], f32, tag="srt")
                nc.scalar.activation(srt[:], red[:], FT.Sqrt, scale=1.0 / D,
                                     bias=epsb[:])
                rstd = wp1.tile([P, NQ], f32, tag="rstd")
                nc.vector.reciprocal(rstd[:], srt[:])
                qn = wp1.tile([P, DQ], f16, tag="qn")
                qn3 = qn[:].rearrange("p (h d) -> p h d", d=D)
                nc.vector.tensor_mul(qn3,
                                     pa[:, 0:512].rearrange("p (h d) -> p h d", d=D),
                                     rstd[:].unsqueeze(-1).broadcast_to([P, NQ, D]))
                kred = wp1.tile([P, 2], f32, tag="kred")
                nc.vector.reduce_sum(kred[:].unsqueeze(-1),
                                     sq[:, 512:640].rearrange("p (h d) -> p h d", d=D),
                                     axis=AX.X)
                ksrt = wp1.tile([P, 2], f32, tag="ksrt")
                nc.scalar.activation(ksrt[:], kred[:], FT.Sqrt, scale=1.0 / D,
                                     bias=epsb[:])
                krstd = wp1.tile([P, 2], f32, tag="krstd")
                nc.vector.reciprocal(krstd[:], ksrt[:])
                kn = wp1.tile([P, 128], f16, tag="kn")
                kn3 = kn[:].rearrange("p (h d) -> p h d", d=D)
                nc.vector.tensor_mul(kn3,
                                     pa[:, 512:640].rearrange("p (h d) -> p h d", d=D),
                                     krstd[:].unsqueeze(-1).broadcast_to([P, 2, D]))
                # rope q: full-width f16 tables (cos/sin pre-tiled per head)
                cosq8 = ropeq_sb[:, tt * 1024:tt * 1024 + 512]
                sinq8 = ropeq_sb[:, tt * 1024 + 512:tt * 1024 + 1024]
                sinq83 = sinq8.rearrange("p (h d) -> p h d", d=D)
                tcos = wp1.tile([P, DQ], f16, tag="tcos")
                nc.vector.tensor_mul(tcos[:], qn[:], cosq8)
                rp = wp1.tile([P, DQ], f16, tag="rp")
                rp3 = rp[:].rearrange("p (h d) -> p h d", d=D)
                nc.vector.tensor_mul(rp3[:, :, 0:32], qn3[:, :, 32:64],
                                     sinq83[:, :, 0:32])
                nc.vector.tensor_mul(rp3[:, :, 32:64], qn3[:, :, 0:32],
                                     sinq83[:, :, 32:64])
                qrope = tcos
                nc.vector.tensor_add(qrope[:], tcos[:], rp[:])
                # rope k: narrow table, broadcast over the 2 kv heads
                cosk = ropek_sb[:, tt * 128:tt * 128 + 64]
                sink = ropek_sb[:, tt * 128 + 64:tt * 128 + 128]
                ktcos = wp1.tile([P, 128], f16, tag="ktcos")
                nc.vector.tensor_mul(ktcos[:].rearrange("p (h d) -> p h d", d=D),
                                     kn3,
                                     cosk.unsqueeze(1).broadcast_to([P, 2, D]))
                krp = wp1.tile([P, 128], f16, tag="krp")
                krp3 = krp[:].rearrange("p (h d) -> p h d", d=D)
                nc.vector.tensor_mul(krp3[:, :, 0:32], kn3[:, :, 32:64],
                                     sink[:, 0:32].unsqueeze(1).broadcast_to([P, 2, 32]))
                nc.vector.tensor_mul(krp3[:, :, 32:64], kn3[:, :, 0:32],
                                     sink[:, 32:64].unsqueeze(1).broadcast_to([P, 2, 32]))
                krope = ktcos
                nc.vector.tensor_add(krope[:], ktcos[:], krp[:])
                ropes[tt] = (qrope, krope)

            def emit_tr(tt):
                # hardware XBAR DMA transposes: no PE or DVE involvement
                qrope, krope = ropes.pop(tt)
                for c in range(4):
                    nc.sync.dma_start_transpose(
                        qT[:, tt * 512 + c * P:tt * 512 + (c + 1) * P],
                        qrope[:, c * P:(c + 1) * P])
                nc.sync.dma_start_transpose(kT[:, tt * P:(tt + 1) * P],
                                            krope[:])

            # ---------- phases 2+3 job machinery ----------
            DEPTH = 2

            class HPair:
                pass

            def emit_S(hp, j):
                i, c = hp.i, hp.c
                pa2 = ps.tile([P, 1024], f32, tag="a")
                pt = ptp.tile([P, 1024], f16, tag="pt")
                rel = max(0, (j - 4 * i) * P)
                diag = (j >= 4 * i)
                # cols [0:rel) are fully-masked queries: skip them in the S
                # matmul (rel is 128-aligned -> clean tt-subtile slice); the
                # psum there stays stale, and exp's output for those cols is
                # garbage that PV never reads
                for s in range(2):
                    nc.tensor.matmul(
                        pa2[:, s * 512 + rel:(s + 1) * 512],
                        kT[s * D:(s + 1) * D, j * P:(j + 1) * P],
                        qTv[s * D:(s + 1) * D, 4 * i + rel // P:4 * i + 4, c, :],
                        start=True, stop=True)
                nc.scalar.activation(pt[:], pa2[:], FT.Exp, scale=0.125,
                                     bias=lnb[:])
                if diag:
                    for s in range(2):
                        dst = pt[:, s * 512 + rel:s * 512 + rel + P]
                        nc.vector.tensor_mul(dst, dst, tri[:])
                hp.pts[j] = (pt, rel)

            def emit_PV(hp, j):
                i = hp.i
                pt, rel = hp.pts.pop(j)
                nchunks = 4 * (i + 1)
                for s in range(2):
                    nc.tensor.matmul(
                        hp.po[s][:, rel:BLK],
                        vsb[j][:, s * 65:s * 65 + 65],
                        pt[:, s * 512 + rel:(s + 1) * 512],
                        start=(j == 0), stop=(j == nchunks - 1))
                if j == nchunks - 1:
                    pending_norm.append(hp)

            def emit_normA(hp):
                # DVE half: 1/rowsum for both heads (PE keeps running S jobs)
                hp.rinvs = []
                for s in range(2):
                    po = hp.po[s]
                    rinv = wp.tile([65, BLK], f32, tag="rinv")
                    # approx_fast requires base_partition 0: rows 0:64 produce
                    # unused junk, row 64 is 1/rowsum
                    nc.vector.reciprocal_approx_fast(rinv[:], po[0:65, :])
                    rinv16 = wp.tile([65, BLK], f16, tag="rinv16")
                    nc.vector.tensor_copy(rinv16[64:65, :], rinv[64:65, :])
                    hp.rinvs.append(rinv16)

            def emit_normB(hp):
                # PE broadcast + normalize multiplies, one job after normA
                norms_done[hp.i] += 1
                for s in range(2):
                    po = hp.po[s]
                    rinv16 = hp.rinvs[s]
                    pb = ps.tile([64, BLK], f32, tag="a")
                    nc.tensor.matmul(pb[:], ones[64:65, 0:64],
                                     rinv16[64:65, :], start=True, stop=True)
                    pbs = wp.tile([64, BLK], f16, tag="pbs")
                    nc.vector.tensor_copy(pbs[:], pb[:])
                    if s == 0:
                        nc.vector.tensor_mul(hp.ob[0:64, :], po[0:64, :], pbs[:])
                    else:
                        scr = wp.tile([64, BLK], f16, tag="scr")
                        nc.vector.tensor_mul(scr[:], po[0:64, :], pbs[:])
                        nc.sync.dma_start(hp.ob[64:128, :], scr[:])

            def emit_wo(i, tl, obufs):
                tt = i * 4 + tl
                pA0 = ps.tile([P, 1024], f32, tag="a")
                pA1 = ps.tile([P, 1024], f32, tag="a")
                for c in range(4):
                    lhs = obufs[c][:, tl * P:(tl + 1) * P]
                    for h4 in range(4):
                        dst = (pA0 if h4 < 2 else pA1)
                        nc.tensor.matmul(dst[:, (h4 % 2) * 512:(h4 % 2 + 1) * 512],
                                         lhs, wo_sb[c][:, h4 * 512:(h4 + 1) * 512],
                                         start=(c == 0), stop=(c == 3))
                osb0 = wp.tile([P, 1024], f16, tag="osb")
                nc.vector.tensor_copy(osb0[:], pA0[:])
                nc.sync.dma_start(out_d[tt * P:(tt + 1) * P, 0:1024], osb0[:])
                osb1 = wp.tile([P, 1024], f16, tag="osb")
                nc.vector.tensor_copy(osb1[:], pA1[:])
                nc.sync.dma_start(out_d[tt * P:(tt + 1) * P, 1024:2048], osb1[:])

            flat = []      # (hpair, j, block) S-jobs in emission order
            block_obufs = {}
            for i in range(NBLK):
                block_obufs[i] = {}
                for c in range(4):
                    ob = obp.tile([P, BLK], f16, tag="ob")
                    block_obufs[i][c] = ob
                    hp = HPair()
                    hp.i, hp.c = i, c
                    hp.ob = ob
                    hp.pts = {}
                    hp.po = None
                    for j in range(4 * (i + 1)):
                        flat.append((hp, j, i))

            state = {"idx": 0, "cur_block": 0, "inject": 0}
            queue = []     # PV jobs awaiting emission (depth pipeline)
            pending_norm = []
            pending_normB = []
            norms_done = [0] * NBLK
            pending_wo = []

            def step_job():
                if state["idx"] >= len(flat):
                    return False
                hp, j, i = flat[state["idx"]]
                state["idx"] += 1
                if i != state["cur_block"]:
                    for tl in range(4):
                        pending_wo.append((state["cur_block"], tl))
                    state["cur_block"] = i
                if hp.po is None:
                    po0 = pso.tile([65, BLK], f32, tag="o")
                    po1 = pso.tile([65, BLK], f32, tag="o")
                    hp.po = [po0, po1]
                emit_S(hp, j)
                queue.append((hp, j))
                if len(queue) > DEPTH:
                    emit_PV(*queue.pop(0))
                # normalizes: DVE half eager (po buffers are scarce; deferring
                # deadlocks the pso WAR chain against in-order engine streams);
                # PE half one job later so PE chews S work during the recips
                if pending_normB:
                    emit_normB(pending_normB.pop(0))
                if pending_norm:
                    hp2 = pending_norm.pop(0)
                    emit_normA(hp2)
                    pending_normB.append(hp2)
                # inject one deferred wo-task every 6 S-jobs once its block's
                # normalizes have all been emitted
                state["inject"] += 1
                if (pending_wo and state["inject"] % 6 == 0
                        and norms_done[pending_wo[0][0]] == 4):
                    wb, tl = pending_wo.pop(0)
                    emit_wo(wb, tl, block_obufs[wb])
                return True

            def run_jobs(n):
                for _ in range(n):
                    step_job()

            # ---------- schedule: dense phase 1, then the job stream ----------
            for t in range(NTT):
                emit_tile(t)
                if t >= 1:
                    emit_tr(t - 1)
            emit_tr(NTT - 1)
            while step_job():
                pass
            while queue:
                emit_PV(*queue.pop(0))
            for hp in pending_norm:
                emit_normA(hp)
                pending_normB.append(hp)
            pending_norm = []
            for hp in pending_normB:
                emit_normB(hp)
            pending_normB = []
            for tl in range(4):
                pending_wo.append((NBLK - 1, tl))
            for wb, tl in pending_wo:
                emit_wo(wb, tl, block_obufs[wb])

    nc.compile()
    return nc


def _host_inputs(x, Wq, Wk, Wv, Wo, q_ln_w, k_ln_w):
    x = np.asarray(x, np.float32)
    Wq = np.asarray(Wq, np.float32)
    Wk = np.asarray(Wk, np.float32)
    Wv = np.asarray(Wv, np.float32)
    Wo = np.asarray(Wo, np.float32)
    q_ln_w = np.asarray(q_ln_w, np.float64)
    k_ln_w = np.asarray(k_ln_w, np.float64)

    inv_freq = 1.0 / (1e6 ** (np.arange(0, D, 2, dtype=np.float64) / D))
    t = np.arange(T, dtype=np.float64)
    freqs = np.outer(t, inv_freq)
    emb = np.concatenate([freqs, freqs], -1)
    cos, sin = np.cos(emb), np.sin(emb)
    rot = (np.arange(D) + 32) % D
    sign = np.where(np.arange(D) < 32, -1.0, 1.0)

    def rope_tab(w, ntile):
        cw = np.tile(w[None, :] * cos, (1, ntile))
        sw = np.tile(sign[None, :] * w[rot][None, :] * sin, (1, ntile))
        return np.concatenate([cw, sw], -1).astype(np.float16)

    ropeq = rope_tab(q_ln_w, 8)   # [T, 1024]
    ropek = rope_tab(k_ln_w, 1)   # [T, 128]
    pp_, gg_ = np.meshgrid(np.arange(P), np.arange(P), indexing="ij")
    mtab = (pp_ <= gg_).astype(np.float16)   # 1 where key<=query

    in_maps = []
    for core in range(8):
        b, g = core // 4, core % 4
        xT = np.ascontiguousarray(x[b].T).astype(np.float16)
        heads = []
        for c in range(4):
            heads += [g * 8 + c, g * 8 + c + 4]
        wqkv = np.ascontiguousarray(np.concatenate(
            [Wq[:, h * D:(h + 1) * D] for h in heads]
            + [Wk[:, g * 128:(g + 1) * 128], Wv[:, g * 128:(g + 1) * 128]],
            axis=1)).astype(np.float16)
        wo = np.ascontiguousarray(
            np.concatenate([Wo[h * D:(h + 1) * D, :] for h in heads], axis=0)
        ).astype(np.float16)
        in_maps.append({
            "xT": xT, "wqkv": wqkv, "wo": wo,
            "ropeq": ropeq, "ropek": ropek, "mtab": mtab,
        })
    return in_maps


def get_program():
    if "nc" not in _CACHE:
        _CACHE["nc"] = _build_program()
    return _CACHE["nc"]


def run(inputs, trace=False, tmpdir=None):
    nc = get_program()
    in_maps = _host_inputs(**inputs)
    res = run_bass_kernel_spmd(nc, in_maps, list(range(8)), trace=trace, tmpdir=tmpdir)
    out = np.zeros((2, T, H), np.float32)
    for core in range(8):
        out[core // 4] += res.results[core]["out"].astype(np.float32)
    return out, res


def kernel(**inputs) -> np.ndarray:
    out, _ = run(inputs, trace=False)
    return out


# revision 40
# speedup vs baseline: 1.0523x; 1.0523x over previous
"""GQA attention block (qk-rmsnorm + RoPE + causal GQA attention + out-proj),
tensor-parallel over 8 NeuronCores: 2-way data parallel (batch) x 4-way head
parallel (8 q heads / 2 kv heads per core). All-reduce of out-proj partials is
done on host (sum of 4 partials per batch).

Per-core layouts (device):
  phase 1: q/k/v projections with x^T chunks stationary on PE -> [T,d] rows;
           qk-rmsnorm + RoPE in row layout (f16); PE-transpose q,k to [d,T].
           Tiles 0-7 run up front; tiles 8-15 are sprinkled into the phase-2
           job stream (block i only needs tiles 0..4i+3) to fill PE stalls.
  phase 2: per (head-chunk c, key chunk j): S^T for BOTH kv heads back-to-back
           (K=64 matmuls on disjoint PE row halves, hardware-concurrent),
           one exp covering both (scale=1/8 + ln(1/64) folded) -> P~ f16;
           causal triangle applied as a 0/1 f16 mask multiply on DVE; PV via
           [V|1] stationary -> out^T + rowsum in one psum; DVE fast-reciprocal
           + f16 PE row-broadcast.
  phase 3: out-proj from packed head pairs, f16 weights, accumulate f32 psum,
           f16 partial outputs summed on host.
"""
import sys
import numpy as np

sys.path.insert(0, "/opt/trn_rl_repo")

import concourse.bass as bass  # noqa: E402
import concourse.bacc as bacc  # noqa: E402
import concourse.mybir as mybir  # noqa: E402
import concourse.tile as tile  # noqa: E402
from concourse import masks  # noqa: E402
from concourse.bass_utils import run_bass_kernel_spmd  # noqa: E402

f32 = mybir.dt.float32
f32r = mybir.dt.float32r
f16 = mybir.dt.float16
FT = mybir.ActivationFunctionType
AX = mybir.AxisListType

P = 128
T = 2048
H = 2048
D = 64
NQ = 8          # q heads per core
DQ = NQ * D     # 512
NTT = T // P    # 16 T tiles
NHC = H // P    # 16 hidden chunks
NBLK = 4        # T_q blocks of 512
BLK = 512
EPS = 1e-5
LN64 = -4.1588830833596715  # ln(1/64): scales exp to keep 1/rowsum in f16 normal range

_CACHE = {}


def _build_program():
    nc = bacc.Bacc("TRN2", target_bir_lowering=False, debug=False, num_devices=8)

    xT_d = nc.dram_tensor("xT", [H, T], f16, kind="ExternalInput")
    wqkv_d = nc.dram_tensor("wqkv", [H, 768], f16, kind="ExternalInput")
    wo_d = nc.dram_tensor("wo", [DQ, H], f16, kind="ExternalInput")
    ropeq_d = nc.dram_tensor("ropeq", [T, 1024], f16, kind="ExternalInput")
    ropek_d = nc.dram_tensor("ropek", [T, 128], f16, kind="ExternalInput")
    mtab_d = nc.dram_tensor("mtab", [P, 128], f16, kind="ExternalInput")
    out_d = nc.dram_tensor("out", [T, H], f16, kind="ExternalOutput")

    with tile.TileContext(nc) as tc:
        with (
            tc.tile_pool(name="persist", bufs=1) as pp,
            tc.tile_pool(name="work2", bufs=2) as wp,
            tc.tile_pool(name="ptp", bufs=3) as ptp,
            tc.tile_pool(name="obp", bufs=8) as obp,
            tc.tile_pool(name="psum", bufs=3, space="PSUM") as ps,
            tc.tile_pool(name="psum_o", bufs=2, space="PSUM") as pso,
            tc.tile_pool(name="ph1", bufs=1) as p1,
            tc.tile_pool(name="work1", bufs=2) as wp1,
        ):
            # ---------- long-lived tiles (wo/tri DMAs issued after xt below:
            # they are not needed until phase 2/3) ----------
            wo_sb = []
            for c in range(4):
                t3 = pp.tile([P, H], f16, tag=f"wo{c}")
                wo_sb.append(t3)
            tri = pp.tile([P, P], f16, tag="tri")   # 1 where key<=query (p<=c)
            ones = pp.tile([P, 65], f16, tag="ones")
            nc.gpsimd.memset(ones[:], 1.0)
            lnb = pp.tile([P, 1], f32, tag="lnb")
            nc.gpsimd.memset(lnb[:], LN64)
            epsb = pp.tile([P, 1], f32, tag="epsb")
            nc.gpsimd.memset(epsb[:], EPS)

            # qT stored tile-major: [d-pair(128), tt(16) x c(4) x t(128)]
            qT = pp.tile([P, NTT * 512], f16, tag="qT")
            kT = pp.tile([P, T], f16, tag="kT")        # kv0 rows 0:64, kv1 rows 64:128
            vsb = []
            for tt in range(NTT):
                vt = pp.tile([P, 130], f16, tag=f"v{tt}")
                nc.gpsimd.memset(vt[:, 64:65], 1.0)     # ones col for kv0
                nc.gpsimd.memset(vt[:, 129:130], 1.0)   # ones col for kv1
                vsb.append(vt)

            qTv = qT[:].rearrange("p (tt c t) -> p tt c t", c=4, t=P)

            # ---------- phase-1 inputs ----------
            wqkv_sb = []
            xt_sb = []
            for hc in range(NHC):
                t1 = p1.tile([P, 768], f16, tag=f"wqkv{hc}")
                nc.sync.dma_start(t1[:], wqkv_d[hc * P:(hc + 1) * P, :])
                wqkv_sb.append(t1)
                tx = p1.tile([P, T], f16, tag=f"xt{hc}")
                nc.sync.dma_start(tx[:], xT_d[hc * P:(hc + 1) * P, :])
                xt_sb.append(tx)
            ropeq_sb = p1.tile([P, 16 * 1024], f16, tag="ropeq")
            ropek_sb = p1.tile([P, 16 * 128], f16, tag="ropek")
            for tt in range(NTT):
                nc.sync.dma_start(ropeq_sb[:, tt * 1024:(tt + 1) * 1024],
                                  ropeq_d[tt * P:(tt + 1) * P, :])
                nc.sync.dma_start(ropek_sb[:, tt * 128:(tt + 1) * 128],
                                  ropek_d[tt * P:(tt + 1) * P, :])
            ident = p1.tile([P, P], f16, tag="ident")
            masks.make_identity(nc, ident[:])
            for c in range(4):
                nc.sync.dma_start(wo_sb[c][:], wo_d[c * P:(c + 1) * P, :])
            nc.sync.dma_start(tri[:], mtab_d[:])

            ropes = {}   # tt -> (qrope, krope) awaiting transpose

            def emit_tile(tt):
                pa = ps.tile([P, 1024], f32, tag="a")
                for hc in range(NHC):
                    lhs = xt_sb[hc][:, tt * P:(tt + 1) * P]
                    nc.tensor.matmul(pa[:, 0:512], lhs, wqkv_sb[hc][:, 0:512],
                                     start=(hc == 0), stop=(hc == NHC - 1))
                    nc.tensor.matmul(pa[:, 512:768], lhs, wqkv_sb[hc][:, 512:768],
                                     start=(hc == 0), stop=(hc == NHC - 1))
                # v eviction (no norm): two flat copies into the kv slots
                vt = vsb[tt]
                nc.vector.tensor_copy(vt[:, 0:64], pa[:, 640:704])
                nc.vector.tensor_copy(vt[:, 65:129], pa[:, 704:768])
                # q+k rmsnorm (one fused square over q|k columns)
                sq = wp1.tile([P, 640], f32, tag="sq")
                nc.scalar.activation(sq[:], pa[:, 0:640], FT.Square)
                red = wp1.tile([P, NQ], f32, tag="red")
                nc.vector.reduce_sum(red[:].unsqueeze(-1),
                                     sq[:, 0:512].rearrange("p (h d) -> p h d", d=D),
                                     axis=AX.X)
                srt = wp1.tile([P, NQ], f32, tag="srt")
                nc.scalar.activation(srt[:], red[:], FT.Sqrt, scale=1.0 / D,
                                     bias=epsb[:])
                rstd = wp1.tile([P, NQ], f32, tag="rstd")
                nc.vector.reciprocal(rstd[:], srt[:])
                qn = wp1.tile([P, DQ], f16, tag="qn")
                qn3 = qn[:].rearrange("p (h d) -> p h d", d=D)
                nc.vector.tensor_mul(qn3,
                                     pa[:, 0:512].rearrange("p (h d) -> p h d", d=D),
                                     rstd[:].unsqueeze(-1).broadcast_to([P, NQ, D]))
                kred = wp1.tile([P, 2], f32, tag="kred")
                nc.vector.reduce_sum(kred[:].unsqueeze(-1),
                                     sq[:, 512:640].rearrange("p (h d) -> p h d", d=D),
                                     axis=AX.X)
                ksrt = wp1.tile([P, 2], f32, tag="ksrt")
                nc.scalar.activation(ksrt[:], kred[:], FT.Sqrt, scale=1.0 / D,
                                     bias=epsb[:])
                krstd = wp1.tile([P, 2], f32, tag="krstd")
                nc.vector.reciprocal(krstd[:], ksrt[:])
                kn = wp1.tile([P, 128], f16, tag="kn")
                kn3 = kn[:].rearrange("p (h d) -> p h d", d=D)
                nc.vector.tensor_mul(kn3,
                                     pa[:, 512:640].rearrange("p (h d) -> p h d", d=D),
                                     krstd[:].unsqueeze(-1).broadcast_to([P, 2, D]))
                # rope q: full-width f16 tables (cos/sin pre-tiled per head)
                cosq8 = ropeq_sb[:, tt * 1024:tt * 1024 + 512]
                sinq8 = ropeq_sb[:, tt * 1024 + 512:tt * 1024 + 1024]
                sinq83 = sinq8.rearrange("p (h d) -> p h d", d=D)
                tcos = wp1.tile([P, DQ], f16, tag="tcos")
                nc.vector.tensor_mul(tcos[:], qn[:], cosq8)
                rp = wp1.tile([P, DQ], f16, tag="rp")
                rp3 = rp[:].rearrange("p (h d) -> p h d", d=D)
                nc.vector.tensor_mul(rp3[:, :, 0:32], qn3[:, :, 32:64],
                                     sinq83[:, :, 0:32])
                nc.vector.tensor_mul(rp3[:, :, 32:64], qn3[:, :, 0:32],
                                     sinq83[:, :, 32:64])
                qrope = tcos
                nc.vector.tensor_add(qrope[:], tcos[:], rp[:])
                # rope k: narrow table, broadcast over the 2 kv heads
                cosk = ropek_sb[:, tt * 128:tt * 128 + 64]
                sink = ropek_sb[:, tt * 128 + 64:tt * 128 + 128]
                ktcos = wp1.tile([P, 128], f16, tag="ktcos")
                nc.vector.tensor_mul(ktcos[:].rearrange("p (h d) -> p h d", d=D),
                                     kn3,
                                     cosk.unsqueeze(1).broadcast_to([P, 2, D]))
                krp = wp1.tile([P, 128], f16, tag="krp")
                krp3 = krp[:].rearrange("p (h d) -> p h d", d=D)
                nc.vector.tensor_mul(krp3[:, :, 0:32], kn3[:, :, 32:64],
                                     sink[:, 0:32].unsqueeze(1).broadcast_to([P, 2, 32]))
                nc.vector.tensor_mul(krp3[:, :, 32:64], kn3[:, :, 0:32],
                                     sink[:, 32:64].unsqueeze(1).broadcast_to([P, 2, 32]))
                krope = ktcos
                nc.vector.tensor_add(krope[:], ktcos[:], krp[:])
                ropes[tt] = (qrope, krope)

            def emit_tr(tt):
                qrope, krope = ropes.pop(tt)
                ptrk = ps.tile([P, 640], f16, tag="a")
                for c in range(4):
                    nc.tensor.transpose(ptrk[:, c * P:(c + 1) * P],
                                        qrope[:, c * P:(c + 1) * P], ident[:])
                nc.tensor.transpose(ptrk[:, 512:640], krope[:], ident[:])
                nc.vector.tensor_copy(qT[:, tt * 512:(tt + 1) * 512],
                                      ptrk[:, 0:512])
                nc.vector.tensor_copy(kT[:, tt * P:(tt + 1) * P],
                                      ptrk[:, 512:640])

            # ---------- phases 2+3 job machinery ----------
            DEPTH = 2

            class HPair:
                pass

            def emit_S(hp, j):
                i, c = hp.i, hp.c
                pa2 = ps.tile([P, 1024], f32, tag="a")
                pt = ptp.tile([P, 1024], f16, tag="pt")
                rel = max(0, (j - 4 * i) * P)
                diag = (j >= 4 * i)
                # cols [0:rel) are fully-masked queries: skip them in the S
                # matmul (rel is 128-aligned -> clean tt-subtile slice); the
                # psum there stays stale, and exp's output for those cols is
                # garbage that PV never reads
                for s in range(2):
                    nc.tensor.matmul(
                        pa2[:, s * 512 + rel:(s + 1) * 512],
                        kT[s * D:(s + 1) * D, j * P:(j + 1) * P],
                        qTv[s * D:(s + 1) * D, 4 * i + rel // P:4 * i + 4, c, :],
                        start=True, stop=True)
                nc.scalar.activation(pt[:], pa2[:], FT.Exp, scale=0.125,
                                     bias=lnb[:])
                if diag:
                    for s in range(2):
                        dst = pt[:, s * 512 + rel:s * 512 + rel + P]
                        nc.vector.tensor_mul(dst, dst, tri[:])
                hp.pts[j] = (pt, rel)

            def emit_PV(hp, j):
                i = hp.i
                pt, rel = hp.pts.pop(j)
                nchunks = 4 * (i + 1)
                for s in range(2):
                    nc.tensor.matmul(
                        hp.po[s][:, rel:BLK],
                        vsb[j][:, s * 65:s * 65 + 65],
                        pt[:, s * 512 + rel:(s + 1) * 512],
                        start=(j == 0), stop=(j == nchunks - 1))
                if j == nchunks - 1:
                    pending_norm.append(hp)

            def emit_normA(hp):
                # DVE half: 1/rowsum for both heads (PE keeps running S jobs)
                hp.rinvs = []
                for s in range(2):
                    po = hp.po[s]
                    rinv = wp.tile([65, BLK], f32, tag="rinv")
                    # approx_fast requires base_partition 0: rows 0:64 produce
                    # unused junk, row 64 is 1/rowsum
                    nc.vector.reciprocal_approx_fast(rinv[:], po[0:65, :])
                    rinv16 = wp.tile([65, BLK], f16, tag="rinv16")
                    nc.vector.tensor_copy(rinv16[64:65, :], rinv[64:65, :])
                    hp.rinvs.append(rinv16)

            def emit_normB(hp):
                # PE broadcast + normalize multiplies, one job after normA
                norms_done[hp.i] += 1
                for s in range(2):
                    po = hp.po[s]
                    rinv16 = hp.rinvs[s]
                    pb = ps.tile([64, BLK], f32, tag="a")
                    nc.tensor.matmul(pb[:], ones[64:65, 0:64],
                                     rinv16[64:65, :], start=True, stop=True)
                    pbs = wp.tile([64, BLK], f16, tag="pbs")
                    nc.vector.tensor_copy(pbs[:], pb[:])
                    if s == 0:
                        nc.vector.tensor_mul(hp.ob[0:64, :], po[0:64, :], pbs[:])
                    else:
                        scr = wp.tile([64, BLK], f16, tag="scr")
                        nc.vector.tensor_mul(scr[:], po[0:64, :], pbs[:])
                        nc.sync.dma_start(hp.ob[64:128, :], scr[:])

            def emit_wo(i, tl, obufs):
                tt = i * 4 + tl
                pA0 = ps.tile([P, 1024], f32, tag="a")
                pA1 = ps.tile([P, 1024], f32, tag="a")
                for c in range(4):
                    lhs = obufs[c][:, tl * P:(tl + 1) * P]
                    for h4 in range(4):
                        dst = (pA0 if h4 < 2 else pA1)
                        nc.tensor.matmul(dst[:, (h4 % 2) * 512:(h4 % 2 + 1) * 512],
                                         lhs, wo_sb[c][:, h4 * 512:(h4 + 1) * 512],
                                         start=(c == 0), stop=(c == 3))
                osb0 = wp.tile([P, 1024], f16, tag="osb")
                nc.vector.tensor_copy(osb0[:], pA0[:])
                nc.sync.dma_start(out_d[tt * P:(tt + 1) * P, 0:1024], osb0[:])
                osb1 = wp.tile([P, 1024], f16, tag="osb")
                nc.vector.tensor_copy(osb1[:], pA1[:])
                nc.sync.dma_start(out_d[tt * P:(tt + 1) * P, 1024:2048], osb1[:])

            flat = []      # (hpair, j, block) S-jobs in emission order
            block_obufs = {}
            for i in range(NBLK):
                block_obufs[i] = {}
                for c in range(4):
                    ob = obp.tile([P, BLK], f16, tag="ob")
                    block_obufs[i][c] = ob
                    hp = HPair()
                    hp.i, hp.c = i, c
                    hp.ob = ob
                    hp.pts = {}
                    hp.po = None
                    for j in range(4 * (i + 1)):
                        flat.append((hp, j, i))

            state = {"idx": 0, "cur_block": 0, "inject": 0}
            queue = []     # PV jobs awaiting emission (depth pipeline)
            pending_norm = []
            pending_normB = []
            norms_done = [0] * NBLK
            pending_wo = []

            def step_job():
                if state["idx"] >= len(flat):
                    return False
                hp, j, i = flat[state["idx"]]
                state["idx"] += 1
                if i != state["cur_block"]:
                    for tl in range(4):
                        pending_wo.append((state["cur_block"], tl))
                    state["cur_block"] = i
                if hp.po is None:
                    po0 = pso.tile([65, BLK], f32, tag="o")
                    po1 = pso.tile([65, BLK], f32, tag="o")
                    hp.po = [po0, po1]
                emit_S(hp, j)
                queue.append((hp, j))
                if len(queue) > DEPTH:
                    emit_PV(*queue.pop(0))
                # normalizes: DVE half eager (po buffers are scarce; deferring
                # deadlocks the pso WAR chain against in-order engine streams);
                # PE half one job later so PE chews S work during the recips
                if pending_normB:
                    emit_normB(pending_normB.pop(0))
                if pending_norm:
                    hp2 = pending_norm.pop(0)
                    emit_normA(hp2)
                    pending_normB.append(hp2)
                # inject one deferred wo-task every 6 S-jobs once its block's
                # normalizes have all been emitted
                state["inject"] += 1
                if (pending_wo and state["inject"] % 6 == 0
                        and norms_done[pending_wo[0][0]] == 4):
                    wb, tl = pending_wo.pop(0)
                    emit_wo(wb, tl, block_obufs[wb])
                return True

            def run_jobs(n):
                for _ in range(n):
                    step_job()

            # ---------- schedule: dense phase 1, then the job stream ----------
            for t in range(NTT):
                emit_tile(t)
                if t >= 1:
                    emit_tr(t - 1)
            emit_tr(NTT - 1)
            while step_job():
                pass
            while queue:
                emit_PV(*queue.pop(0))
            for hp in pending_norm:
                emit_normA(hp)
                pending_normB.append(hp)
            pending_norm = []
            for hp in pending_normB:
                emit_normB(hp)
            pending_normB = []
            for tl in range(4):
                pending_wo.append((NBLK - 1, tl))
            for wb, tl in pending_wo:
                emit_wo(wb, tl, block_obufs[wb])

    nc.compile()
    return nc


def _host_inputs(x, Wq, Wk, Wv, Wo, q_ln_w, k_ln_w):
    x = np.asarray(x, np.float32)
    Wq = np.asarray(Wq, np.float32)
    Wk = np.asarray(Wk, np.float32)
    Wv = np.asarray(Wv, np.float32)
    Wo = np.asarray(Wo, np.float32)
    q_ln_w = np.asarray(q_ln_w, np.float64)
    k_ln_w = np.asarray(k_ln_w, np.float64)

    inv_freq = 1.0 / (1e6 ** (np.arange(0, D, 2, dtype=np.float64) / D))
    t = np.arange(T, dtype=np.float64)
    freqs = np.outer(t, inv_freq)
    emb = np.concatenate([freqs, freqs], -1)
    cos, sin = np.cos(emb), np.sin(emb)
    rot = (np.arange(D) + 32) % D
    sign = np.where(np.arange(D) < 32, -1.0, 1.0)

    def rope_tab(w, ntile):
        cw = np.tile(w[None, :] * cos, (1, ntile))
        sw = np.tile(sign[None, :] * w[rot][None, :] * sin, (1, ntile))
        return np.concatenate([cw, sw], -1).astype(np.float16)

    ropeq = rope_tab(q_ln_w, 8)   # [T, 1024]
    ropek = rope_tab(k_ln_w, 1)   # [T, 128]
    pp_, gg_ = np.meshgrid(np.arange(P), np.arange(P), indexing="ij")
    mtab = (pp_ <= gg_).astype(np.float16)   # 1 where key<=query

    in_maps = []
    for core in range(8):
        b, g = core // 4, core % 4
        xT = np.ascontiguousarray(x[b].T).astype(np.float16)
        heads = []
        for c in range(4):
            heads += [g * 8 + c, g * 8 + c + 4]
        wqkv = np.ascontiguousarray(np.concatenate(
            [Wq[:, h * D:(h + 1) * D] for h in heads]
            + [Wk[:, g * 128:(g + 1) * 128], Wv[:, g * 128:(g + 1) * 128]],
            axis=1)).astype(np.float16)
        wo = np.ascontiguousarray(
            np.concatenate([Wo[h * D:(h + 1) * D, :] for h in heads], axis=0)
        ).astype(np.float16)
        in_maps.append({
            "xT": xT, "wqkv": wqkv, "wo": wo,
            "ropeq": ropeq, "ropek": ropek, "mtab": mtab,
        })
    return in_maps


def get_program():
    if "nc" not in _CACHE:
        _CACHE["nc"] = _build_program()
    return _CACHE["nc"]


def run(inputs, trace=False, tmpdir=None):
    nc = get_program()
    in_maps = _host_inputs(**inputs)
    res = run_bass_kernel_spmd(nc, in_maps, list(range(8)), trace=trace, tmpdir=tmpdir)
    out = np.zeros((2, T, H), np.float32)
    for core in range(8):
        out[core // 4] += res.results[core]["out"].astype(np.float32)
    return out, res


def kernel(**inputs) -> np.ndarray:
    out, _ = run(inputs, trace=False)
    return out
